# revision 6
# baseline (speedup 1.0000x reference)
"""TT (tensor-train) AdaptiveRankLinear forward on 8 TRN2 NeuronCores.

Strategy: the TT weight (g1,g2,g3) is tiny (~1 MB) and static, so we
materialize the dense W = TT-contract(g1,g2,g3) on the host (~0.6 GFLOP)
and run the remaining 137 GFLOP x @ W + bias as a data-parallel dense
matmul: batch sharded 8 ways, W (bf16) replicated, each core computing a
512x4096 @ 4096x4096 bf16 matmul with f32 PSUM accumulation.

W and x^T are pre-tiled on the host into the exact [k_tile][n_tile]
[partition][k_subtile][col] blocks the kernel consumes, so every SBUF
tile load is one contiguous 512KB DMA with 4KB-per-partition runs.
"""

import sys

sys.path.insert(0, "/opt/trn_rl_repo")

import numpy as np
import ml_dtypes

B = 4096
D_IN = 4096
D_OUT = 4096
N_CORES = 8
BS = B // N_CORES  # 512 rows per core

P = 128
K_TILE = 512
K_SUB = K_TILE // P  # 4
K_TILES = D_IN // K_TILE  # 8
N_TILE = 512
N_TILES = D_OUT // N_TILE  # 8

_CACHE = {}


def _get_nc():
    if "nc" in _CACHE:
        return _CACHE["nc"]

    import concourse.mybir as mybir
    import concourse.tile as tile
    from concourse import bacc
    from concourse.kernels.tile_matmul import (
        ShapeInfo,
        composable_matmul_tile_kernel,
    )

    nc = bacc.Bacc(None, target_bir_lowering=False)
    # pre-tiled layouts: xT[ko][pi][s][m], w[ko][no][pi][s][c]
    xT = nc.declare_dram_parameter(
        "xT", [K_TILES, P, K_SUB, BS], mybir.dt.bfloat16, isOutput=False
    )
    w = nc.declare_dram_parameter(
        "w", [K_TILES, N_TILES, P, K_SUB, N_TILE], mybir.dt.bfloat16, isOutput=False
    )
    biasr = nc.declare_dram_parameter(
        "biasr", [P, D_OUT], mybir.dt.float32, isOutput=False
    )
    out = nc.declare_dram_parameter("out", [BS, D_OUT], mybir.dt.float32, isOutput=True)

    with tile.TileContext(nc) as tc:
        with (
            tc.tile_pool(name="const", bufs=1) as const_pool,
            tc.tile_pool(name="kxm_pool", bufs=K_TILES + 1) as kxm_pool,
            tc.tile_pool(name="kxn_pool", bufs=4) as kxn_pool,
        ):
            bias_sb = const_pool.tile([P, D_OUT], mybir.dt.float32)
            out_t = out[:].rearrange("(po pi) f -> pi po f", pi=P)
            bias_loaded = [False]

            def kxm_producer(nc_, md):
                t = kxm_pool.tile([P, K_SUB, BS], mybir.dt.bfloat16, tag="kxm")
                nc_.sync.dma_start(t[:], xT[md.k_tile_idx])
                return t[:]

            def kxn_producer(nc_, md):
                t = kxn_pool.tile([P, K_SUB, N_TILE], mybir.dt.bfloat16, tag="kxn")
                nc_.sync.dma_start(t[:], w[md.k_tile_idx, md.n_tile_idx])
                return t[:]

            def bias_reducer(nc_, psum, sbuf, md):
                # Lazy bias load: issued at the first eviction (~25us in) so
                # its 2MB of DMA descriptors don't contend with the first
                # W/x tile loads that gate the first matmul.
                if not bias_loaded[0]:
                    nc_.gpsimd.dma_start(bias_sb[:], biasr[:])
                    bias_loaded[0] = True
                # psum -> sbuf eviction fused with the bias add, then DMA the
                # subtile out immediately (the tail only waits on 256KB, not
                # the whole 1MB mxn tile)
                sz = md.n_subtile_slice_size
                s = md.n_tile_idx * md.n_tile + md.n_subtile_idx * md.n_subtile
                nc_.vector.tensor_add(
                    out=sbuf[:, :, :sz],
                    in0=psum[:, :sz],
                    in1=bias_sb[: sbuf.shape[0], s : s + sz],
                )
                po = md.m_tile_idx * md.m_subtiles + md.m_subtile_idx
                nc_.sync.dma_start(
                    out_t[:, po : po + 1, s : s + sz], sbuf[:, :, :sz]
                )

            def mxn_consumer(nc_, mxn_tile, md):
                pass  # subtiles are written out by bias_reducer

            kxm_shape = ShapeInfo(pdims=((P, D_IN // P),), fdims=(BS,))
            kxn_shape = ShapeInfo(pdims=((P, D_IN // P),), fdims=(D_OUT,))
            composable_matmul_tile_kernel(
                tc=tc,
                kxm_shape=kxm_shape,
                kxn_shape=kxn_shape,
                output_type=mybir.dt.float32,
                kxm_producer=kxm_producer,
                kxn_producer=kxn_producer,
                mxn_consumer=mxn_consumer,
                mxn_subtile_reducer=bias_reducer,
                psum_n_bufs=2,
                MAX_K_TILE_SIZE=K_TILE,
            )
    nc.compile()
    _CACHE["nc"] = nc
    return nc


def _materialize_w(g1, g2, g3):
    # W[(i j k), (n p q)] = sum_{r,s} g1[i,n,r] g2[r,j,p,s] g3[s,k,q]
    W = np.einsum(
        "inr,rjps,skq->ijknpq",
        np.asarray(g1, np.float32),
        np.asarray(g2, np.float32),
        np.asarray(g3, np.float32),
        optimize=True,
    )
    return np.ascontiguousarray(W.reshape(D_IN, D_OUT))


def _make_in_maps(x, g1, g2, g3, bias):
    W = _materialize_w(g1, g2, g3)
    Wb = W.astype(ml_dtypes.bfloat16)
    # [k, n] -> [ko, no, pi, s, c]: row k = ko*K_TILE + s*P + pi
    Wt = np.ascontiguousarray(
        Wb.reshape(K_TILES, K_SUB, P, N_TILES, N_TILE).transpose(0, 3, 2, 1, 4)
    )
    biasr = np.ascontiguousarray(
        np.broadcast_to(np.asarray(bias, np.float32), (P, D_OUT))
    )
    xb = np.asarray(x, np.float32).astype(ml_dtypes.bfloat16)
    in_maps = []
    for c in range(N_CORES):
        xT = xb[c * BS : (c + 1) * BS, :].T  # [K, BS]
        xTt = np.ascontiguousarray(
            xT.reshape(K_TILES, K_SUB, P, BS).transpose(0, 2, 1, 3)
        )
        in_maps.append({"xT": xTt, "w": Wt, "biasr": biasr})
    return in_maps


def _run(in_maps, trace=False):
    from concourse.bass_utils import run_bass_kernel_spmd

    nc = _get_nc()
    return run_bass_kernel_spmd(nc, in_maps, core_ids=list(range(N_CORES)), trace=trace)


def kernel(x, g1, g2, g3, bias):
    in_maps = _make_in_maps(x, g1, g2, g3, bias)
    res = _run(in_maps)
    out = np.concatenate(
        [res.results[c]["out"] for c in range(N_CORES)], axis=0
    ).astype(np.float32, copy=False)
    return out
